# revision 1
# baseline (speedup 1.0000x reference)
"""Trainium2 Bass kernel for epipolar cross-attention (sparse_attention).

Strategy
--------
The reference gathers, per query pixel l, up to C=240 candidate source
pixels lying in a 5-pixel-wide band around l's epipolar line, and runs
masked softmax attention over them.  Key identity: the candidate set is
exactly  {s : |a_l*x_s + b_l*y_s + c_l| < 2*max(|a_l|,|b_l|)}  with
(a,b,c) the normalized epipolar line -- a rank-3 predicate.  So instead
of gathering, we run *dense banded attention*: per 128-query tile the
union of bands is a contiguous window of source pixels (the geometry
here is near-horizontal translation), and the exact mask is recomputed
on-device from a tiny K=3 GEMM plus one fused compare instruction.

Sharding: queries (L=2304) are split over 8 cores (288 each = exactly 6
image rows).  Each core receives only the source row-range its windows
touch (padded with sentinel rows so every core runs the identical
program).  K^T, V are computed per-core on that range; scores are
computed directly in [s, l] (transposed) orientation so the softmax'd
weights feed the PV matmul without any transposes; the softmax row-sum
rides along as a 33rd `ones` row of the V operand; exp() needs no
max-subtraction (|scores| <= ~6.5 for this data).  Head channels are
de-interleaved host-side (c' = h*32+i) so each head is a contiguous
32-partition slab.
"""

import math

import numpy as np

D = 256
NH = 8
DIM = 32
HH = 48
WW = 48
SCALE = 8
S = HH * WW          # 2304 source pixels
L = S                # 2304 query pixels
NCORES = 8
LC = L // NCORES     # 288 queries per core = 6 image rows
ROWS_PER_CORE = LC // WW  # 6
LTILES = [(0, 128), (128, 128), (256, 32)]
ATILES = [(0, 144), (144, 144)]
LN_EPS = 1e-5
INV_SQRT_DIM = 1.0 / math.sqrt(DIM)

_CACHE: dict = {}


def _host_geometry(K0, K1, R, t):
    """fp32 mirror of reference._candidate_index's line computation."""
    sc = np.float32(SCALE)
    K0s = K0.copy()
    K0s[:, :2, :] = K0s[:, :2, :] / sc
    K1s = K1.copy()
    K1s[:, :2, :] = K1s[:, :2, :] / sc
    gy, gx = np.meshgrid(np.arange(HH), np.arange(WW), indexing="ij")
    coord = np.stack([gx, gy], -1).reshape(S, 2).astype(np.float32)
    coord_h = np.concatenate([coord, np.ones((S, 1), np.float32)], -1)
    tx, ty, tz = t[:, 0, 0], t[:, 1, 0], t[:, 2, 0]
    z = np.zeros_like(tx)
    skew = np.stack(
        [
            np.stack([z, -tz, ty], -1),
            np.stack([tz, z, -tx], -1),
            np.stack([-ty, tx, z], -1),
        ],
        1,
    )
    F = np.swapaxes(np.linalg.inv(K1s), 1, 2) @ skew @ R @ np.linalg.inv(K0s)
    lines = np.einsum("nij,sj->nsi", F, coord_h)[0].astype(np.float32)
    lines = lines / (np.linalg.norm(lines[:, :2], axis=-1, keepdims=True) + 1e-8)
    thr = 2.0 * np.maximum(np.abs(lines[:, 0]), np.abs(lines[:, 1]))
    lines_scaled = (lines / thr[:, None]).astype(np.float32)  # |l . coord| < 1
    return lines_scaled, coord_h


def _plan_windows(lines_scaled, coord_h):
    """Per-ltile source windows, uniform across cores in row-relative terms."""
    mask = np.abs(lines_scaled @ coord_h.T) < 1.0  # [L, S]
    a = [10**9] * len(ATILES)
    b = [-(10**9)] * len(ATILES)
    for c in range(NCORES):
        for i, (tl0, tsz) in enumerate(ATILES):
            gl0 = c * LC + tl0
            cols = np.where(mask[gl0 : gl0 + tsz].any(0))[0]
            lo_row = int(cols.min()) // WW
            hi_row = (int(cols.max()) // WW) + 1
            a[i] = min(a[i], lo_row - ROWS_PER_CORE * c)
            b[i] = max(b[i], hi_row - ROWS_PER_CORE * c)
    A = min(a)
    B = max(b)
    # pad total rows so SR is a multiple of 128 (sentinel rows mask to zero)
    while ((B - A) * WW) % 128 != 0:
        B += 1
    SR = (B - A) * WW
    wins = []
    for i in range(len(ATILES)):
        lo0 = (a[i] - A) * WW
        hi0 = (b[i] - A) * WW
        lo = (lo0 // 128) * 128  # 128-aligned so windows index whole V s-tiles
        wt = -(-(hi0 - lo) // 128) * 128
        lo = min(lo, SR - wt)
        wins.append((lo, wt))
    # containment check of the true mask inside the planned windows
    for c in range(NCORES):
        base = (ROWS_PER_CORE * c + A) * WW
        for i, (tl0, tsz) in enumerate(ATILES):
            gl0 = c * LC + tl0
            cols = np.where(mask[gl0 : gl0 + tsz].any(0))[0]
            lo, wt = wins[i]
            assert int(cols.min()) - base >= lo, (c, i)
            assert int(cols.max()) - base < lo + wt, (c, i)
    return A, B, SR, wins


def _build_program(SR, wins):
    import concourse.bass as bass
    import concourse.mybir as mybir
    from concourse import bacc
    from concourse.tile import TileContext

    fp32 = mybir.dt.float32
    fpr = mybir.dt.float32r
    Alu = mybir.AluOpType
    Act = mybir.ActivationFunctionType
    ST = SR // 128

    nc = bacc.Bacc("TRN2", target_bir_lowering=False)

    xs_d = nc.dram_tensor("xs", [LC, D], fp32, kind="ExternalInput")
    src_d = nc.dram_tensor("srcpad", [SR, D], fp32, kind="ExternalInput")
    lin_d = nc.dram_tensor("linesS", [3, LC], fp32, kind="ExternalInput")
    crd_d = nc.dram_tensor("coordT", [3, SR], fp32, kind="ExternalInput")
    qw_d = nc.dram_tensor("qw", [D, D], fp32, kind="ExternalInput")
    kw_d = nc.dram_tensor("kw", [D, D], fp32, kind="ExternalInput")
    vw_d = nc.dram_tensor("vw", [D, D], fp32, kind="ExternalInput")
    mw_d = nc.dram_tensor("mw", [D, D], fp32, kind="ExternalInput")
    w1_d = nc.dram_tensor("w1", [2 * D, 2 * D], fp32, kind="ExternalInput")
    w2_d = nc.dram_tensor("w2", [2 * D, D], fp32, kind="ExternalInput")
    g1_d = nc.dram_tensor("g1", [1, D], fp32, kind="ExternalInput")
    b1_d = nc.dram_tensor("b1", [1, D], fp32, kind="ExternalInput")
    g2_d = nc.dram_tensor("g2", [1, D], fp32, kind="ExternalInput")
    b2_d = nc.dram_tensor("b2", [1, D], fp32, kind="ExternalInput")
    id_d = nc.dram_tensor("ident", [128, 128], fp32, kind="ExternalInput")
    y_d = nc.dram_tensor("y", [LC, D], fp32, kind="ExternalOutput")

    def bcast_row(ap, p=128):
        # DRAM [1, N] -> broadcast over p partitions for a DMA
        return bass.AP(tensor=ap.tensor, offset=ap.offset, ap=[[0, p]] + ap.ap[1:])

    with TileContext(nc) as tc:
        with (
            tc.tile_pool(name="const", bufs=1) as const,
            tc.tile_pool(name="state", bufs=1) as state,
            tc.tile_pool(name="stage", bufs=1) as stage,
            tc.tile_pool(name="maskp", bufs=2) as maskp,
            tc.tile_pool(name="attnp", bufs=3) as attnp,
            tc.tile_pool(name="small", bufs=4) as small,
            tc.tile_pool(name="work", bufs=3) as work,
            tc.tile_pool(name="ps_big", bufs=4, space="PSUM") as ps_big,
            tc.tile_pool(name="ps_med", bufs=2, space="PSUM") as ps_med,
            tc.tile_pool(name="ps_pv", bufs=2, space="PSUM") as ps_pv,
        ):
            # ---------------- constant loads ----------------
            ident = const.tile([128, 128], fp32, tag="ident")
            nc.gpsimd.dma_start(out=ident, in_=id_d[:, :])
            lin_sb = const.tile([3, LC], fp32, tag="lin")
            nc.gpsimd.dma_start(out=lin_sb, in_=lin_d[:, :])
            crd_sb = const.tile([3, SR], fp32, tag="crd")
            nc.gpsimd.dma_start(out=crd_sb, in_=crd_d[:, :])

            qw_sb = const.tile([128, 2, D], fp32, tag="qw")
            kw_sb = const.tile([128, 2, D], fp32, tag="kw")
            vw_sb = const.tile([128, 2, D], fp32, tag="vw")
            mw_sb = const.tile([128, 2, D], fp32, tag="mw")
            for w_sb, w_d in ((qw_sb, qw_d), (kw_sb, kw_d), (vw_sb, vw_d), (mw_sb, mw_d)):
                nc.sync.dma_start(
                    out=w_sb, in_=w_d.rearrange("(ch p) c -> p ch c", p=128)
                )
            w1_sb = const.tile([128, 4, 2 * D], fp32, tag="w1")
            nc.sync.dma_start(out=w1_sb, in_=w1_d.rearrange("(ch p) c -> p ch c", p=128))
            w2_sb = const.tile([128, 4, D], fp32, tag="w2")
            nc.sync.dma_start(out=w2_sb, in_=w2_d.rearrange("(ch p) c -> p ch c", p=128))

            gb_sb = {}
            for nm, dd in (("g1", g1_d), ("b1", b1_d), ("g2", g2_d), ("b2", b2_d)):
                tile = const.tile([128, D], fp32, tag=nm)
                nc.gpsimd.dma_start(out=tile, in_=bcast_row(dd[:, :]))
                gb_sb[nm] = tile
            eps_sb = const.tile([128, 1], fp32, tag="eps")
            nc.vector.memset(eps_sb, LN_EPS)
            ones_sb = const.tile([128, 128], fp32, tag="ones")
            nc.vector.memset(ones_sb, 1.0)

            xs_sb = const.tile([128, 3, D], fp32, tag="xs")
            for i, (tl0, tsz) in enumerate(LTILES):
                nc.sync.dma_start(
                    out=xs_sb[0:tsz, i, :], in_=xs_d[tl0 : tl0 + tsz, :]
                )
            src_sb = stage.tile([128, ST, D], fp32, tag="src")
            for t in range(ST):
                nc.sync.dma_start(
                    out=src_sb[:, t, :],
                    in_=src_d[t * 128 : (t + 1) * 128, :],
                )

            # ---------------- transposes: srcT, xT ----------------
            srcT = state.tile([128, 2, SR], fp32, tag="srcT")
            for t in range(ST):
                for ch in range(2):
                    tp = ps_med.tile([128, 512], fp32, tag="med")
                    nc.tensor.transpose(
                        tp[:, 0:128], src_sb[:, t, ch * 128 : (ch + 1) * 128], ident
                    )
                    eng = nc.vector if (t + ch) % 2 == 0 else nc.scalar
                    eng.tensor_copy(
                        out=srcT[:, ch, t * 128 : (t + 1) * 128], in_=tp[:, 0:128]
                    ) if eng is nc.vector else eng.copy(
                        out=srcT[:, ch, t * 128 : (t + 1) * 128], in_=tp[:, 0:128]
                    )

            xT = state.tile([128, 2, LC], fp32, tag="xT")
            for i, (tl0, tsz) in enumerate(LTILES):
                for ch in range(2):
                    tp = ps_med.tile([128, 512], fp32, tag="med")
                    nc.tensor.transpose(
                        tp[:, 0:tsz],
                        xs_sb[0:tsz, i, ch * 128 : (ch + 1) * 128],
                        ident[0:tsz, 0:tsz],
                    )
                    nc.vector.tensor_copy(
                        out=xT[:, ch, tl0 : tl0 + tsz], in_=tp[:, 0:tsz]
                    )
            # ---------------- projections ----------------
            # kT[c', s] on the padded source range
            kT = state.tile([128, 2, SR], fp32, tag="kT")
            for ch in range(2):
                off = 0
                while off < SR:
                    n = min(512, SR - off)
                    ps = ps_med.tile([128, 512], fp32, tag="med")
                    for kc in range(2):
                        nc.tensor.matmul(
                            ps[:, 0:n],
                            kw_sb[:, kc, ch * 128 : (ch + 1) * 128],
                            srcT[:, kc, off : off + n],
                            start=(kc == 0),
                            stop=(kc == 1),
                        )
                    nc.scalar.copy(out=kT[:, ch, off : off + n], in_=ps[:, 0:n])
                    off += n
            # vpa[s, h, 0:32] = V, vpa[s, h, 32] = 1 (softmax denominator row)
            vpa = state.tile([128, ST, NH, DIM + 1], fp32, tag="vpa")
            nc.vector.memset(vpa[:, :, :, DIM : DIM + 1], 1.0)
            for t in range(ST):
                ps = ps_med.tile([128, 512], fp32, tag="med")
                for kc in range(2):
                    nc.tensor.matmul(
                        ps[:, 0:D],
                        srcT[:, kc, t * 128 : (t + 1) * 128],
                        vw_sb[:, kc, :],
                        start=(kc == 0),
                        stop=(kc == 1),
                    )
                eng = nc.vector if t % 2 == 0 else nc.gpsimd
                # gpsimd cannot read PSUM; alternate vector/scalar instead
                if t % 2 == 0:
                    nc.vector.tensor_copy(
                        out=vpa[:, t, :, 0:DIM],
                        in_=ps[:, 0:D].rearrange("p (h i) -> p h i", h=NH),
                    )
                else:
                    nc.scalar.copy(
                        out=vpa[:, t, :, 0:DIM],
                        in_=ps[:, 0:D].rearrange("p (h i) -> p h i", h=NH),
                    )

            # ---------------- attention + merge + MLP per l-tile ----------------
            msgT = state.tile([128, 2, LC], fp32, tag="msgT")
            mlT = state.tile([128, 2, LC], fp32, tag="mlT")

            def layer_norm(ps_in, g, b, lsz, out_tile):
                stats = small.tile([128, 6], fp32, tag="stats")
                mv = small.tile([128, 2], fp32, tag="mv")
                nc.vector.bn_stats(out=stats[0:lsz, :], in_=ps_in)
                nc.vector.bn_aggr(out=mv[0:lsz, :], in_=stats[0:lsz, :])
                rstd = small.tile([128, 1], fp32, tag="rstd")
                nc.scalar.activation(
                    out=rstd[0:lsz, :], in_=mv[0:lsz, 1:2], func=Act.Sqrt,
                    bias=eps_sb[0:lsz, :],
                )
                nc.vector.reciprocal(out=rstd[0:lsz, :], in_=rstd[0:lsz, :])
                nc.vector.tensor_scalar(
                    out=out_tile,
                    in0=ps_in,
                    scalar1=mv[0:lsz, 0:1],
                    scalar2=rstd[0:lsz, :],
                    op0=Alu.subtract,
                    op1=Alu.mult,
                )
                nc.gpsimd.tensor_mul(out_tile, out_tile, g[0:lsz, :])
                nc.gpsimd.tensor_add(out_tile, out_tile, b[0:lsz, :])

            # qT[c', l]
            qT = state.tile([128, 2, LC], fp32, tag="qT")
            for ch in range(2):
                ps = ps_med.tile([128, 512], fp32, tag="med")
                for kc in range(2):
                    nc.tensor.matmul(
                        ps[:, 0:LC],
                        qw_sb[:, kc, ch * 128 : (ch + 1) * 128],
                        xT[:, kc, :],
                        start=(kc == 0),
                        stop=(kc == 1),
                    )
                nc.scalar.copy(out=qT[:, ch, :], in_=ps[:, 0:LC])

            # ---- banded attention per l-tile: only the epipolar window of s ----
            wmax = max(wt for _, wt in wins)
            for i, (tl0, tsz) in enumerate(ATILES):
                lo, wt = wins[i]
                nsub = wt // 128
                # mask [s_sub, l] for this tile, shared across heads
                mt = maskp.tile([128, wmax // 128, 144], fp32, tag="mask")
                m2t = maskp.tile([128, wmax // 128, 144], fp32, tag="mask2")
                for sub in range(nsub):
                    dp = ps_med.tile([128, 512], fp32, tag="med")
                    nc.tensor.matmul(
                        dp[:, 0:tsz],
                        crd_sb[:, lo + sub * 128 : lo + (sub + 1) * 128],
                        lin_sb[:, tl0 : tl0 + tsz],
                        start=True,
                        stop=True,
                    )
                    nc.vector.tensor_scalar(
                        out=mt[:, sub, 0:tsz], in0=dp[:, 0:tsz],
                        scalar1=1.0, scalar2=None, op0=Alu.is_lt,
                    )
                    nc.vector.tensor_scalar(
                        out=m2t[:, sub, 0:tsz], in0=dp[:, 0:tsz],
                        scalar1=-1.0, scalar2=None, op0=Alu.is_gt,
                    )
                nc.vector.tensor_mul(
                    mt[:, 0:nsub, 0:tsz], mt[:, 0:nsub, 0:tsz], m2t[:, 0:nsub, 0:tsz]
                )
                for h in range(NH):
                    hp = (h % 4) * 32
                    hc = h // 4
                    at = attnp.tile([128, wmax // 128, 144], fp32, tag="attn")
                    for gs in range(0, nsub, 3):
                        gn = min(3, nsub - gs)
                        sc = ps_big.tile([128, 3, 144], fp32, tag="sc")
                        for k in range(gn):
                            sub = gs + k
                            nc.tensor.matmul(
                                sc[:, k, 0:tsz],
                                kT[hp : hp + 32, hc, lo + sub * 128 : lo + (sub + 1) * 128],
                                qT[hp : hp + 32, hc, tl0 : tl0 + tsz],
                                start=True,
                                stop=True,
                                tile_position=(hp, 0),
                            )
                        nc.scalar.activation(
                            out=at[:, gs : gs + gn, 0:tsz],
                            in_=sc[:, 0:gn, 0:tsz],
                            func=Act.Exp,
                            scale=INV_SQRT_DIM,
                        )
                    meng = nc.gpsimd if (i * NH + h) % 2 == 0 else nc.vector
                    meng.tensor_mul(
                        at[:, 0:nsub, 0:tsz], at[:, 0:nsub, 0:tsz], mt[:, 0:nsub, 0:tsz]
                    )
                    pv = ps_pv.tile([DIM + 1, 144], fp32, tag="pv")
                    for sub in range(nsub):
                        nc.tensor.matmul(
                            pv[:, 0:tsz],
                            vpa[:, lo // 128 + sub, h, :],
                            at[:, sub, 0:tsz],
                            start=(sub == 0),
                            stop=(sub == nsub - 1),
                        )
                    rsh = small.tile([1, 144], fp32, tag="rsh")
                    nc.vector.reciprocal(out=rsh[:, 0:tsz], in_=pv[DIM : DIM + 1, 0:tsz])
                    rs32 = small.tile([DIM, 144], fp32, tag="rs32")
                    nc.gpsimd.partition_broadcast(rs32[:, 0:tsz], rsh[:, 0:tsz])
                    nc.vector.tensor_mul(
                        msgT[hp : hp + 32, hc, tl0 : tl0 + tsz],
                        pv[0:DIM, 0:tsz],
                        rs32[:, 0:tsz],
                    )

            for i, (tl0, tsz) in enumerate(LTILES):
                # merge + LN1
                mg = ps_med.tile([128, 512], fp32, tag="med")
                for kc in range(2):
                    nc.tensor.matmul(
                        mg[0:tsz, 0:D],
                        msgT[:, kc, tl0 : tl0 + tsz],
                        mw_sb[:, kc, :],
                        start=(kc == 0),
                        stop=(kc == 1),
                    )
                mln = work.tile([128, D], fp32, tag="mln")
                layer_norm(mg[0:tsz, 0:D], gb_sb["g1"], gb_sb["b1"], tsz, mln[0:tsz, :])
                for ch in range(2):
                    tp = ps_med.tile([128, 512], fp32, tag="med")
                    nc.tensor.transpose(
                        tp[:, 0:tsz],
                        mln[0:tsz, ch * 128 : (ch + 1) * 128],
                        ident[0:tsz, 0:tsz],
                    )
                    nc.vector.tensor_copy(
                        out=mlT[:, ch, tl0 : tl0 + tsz], in_=tp[:, 0:tsz]
                    )

            # ---------------- MLP (transposed h1 so no transpose needed) ----------------
            h1T = state.tile([128, 4, LC], fp32, tag="h1T")
            for mc in range(4):
                ps = ps_med.tile([128, 512], fp32, tag="med")
                for kc in range(4):
                    rhs = xT[:, kc, :] if kc < 2 else mlT[:, kc - 2, :]
                    nc.tensor.matmul(
                        ps[:, 0:LC],
                        w1_sb[:, kc, mc * 128 : (mc + 1) * 128],
                        rhs,
                        start=(kc == 0),
                        stop=(kc == 3),
                    )
                nc.vector.tensor_scalar_max(h1T[:, mc, :], ps[:, 0:LC], 0.0)

            for i, (tl0, tsz) in enumerate(LTILES):
                m2 = ps_med.tile([128, 512], fp32, tag="med")
                for kc in range(4):
                    nc.tensor.matmul(
                        m2[0:tsz, 0:D],
                        h1T[:, kc, tl0 : tl0 + tsz],
                        w2_sb[:, kc, :],
                        start=(kc == 0),
                        stop=(kc == 3),
                    )
                mo = work.tile([128, D], fp32, tag="mo")
                layer_norm(m2[0:tsz, 0:D], gb_sb["g2"], gb_sb["b2"], tsz, mo[0:tsz, :])
                nc.vector.tensor_add(mo[0:tsz, :], mo[0:tsz, :], xs_sb[0:tsz, i, :])
                nc.gpsimd.dma_start(out=y_d[tl0 : tl0 + tsz, :], in_=mo[0:tsz, :])

    nc.compile()
    return nc


def _prepare(inputs):
    x = np.ascontiguousarray(inputs["x"][0], dtype=np.float32)
    src = np.ascontiguousarray(inputs["source"][0], dtype=np.float32)
    lines_scaled, coord_h = _host_geometry(
        np.asarray(inputs["K0"], np.float32),
        np.asarray(inputs["K1"], np.float32),
        np.asarray(inputs["R"], np.float32),
        np.asarray(inputs["t"], np.float32),
    )
    A, B, SR, wins = _plan_windows(lines_scaled, coord_h)

    perm = np.arange(D).reshape(DIM, NH).T.reshape(-1)  # c' = h*32+i -> i*8+h
    qw = np.ascontiguousarray(np.asarray(inputs["qW"], np.float32)[:, perm])
    kw = np.ascontiguousarray(np.asarray(inputs["kW"], np.float32)[:, perm])
    vw = np.ascontiguousarray(np.asarray(inputs["vW"], np.float32)[:, perm])
    mw = np.ascontiguousarray(np.asarray(inputs["mergeW"], np.float32)[perm, :])

    common = {
        "qw": qw, "kw": kw, "vw": vw, "mw": mw,
        "w1": np.ascontiguousarray(inputs["mlpW1"], dtype=np.float32),
        "w2": np.ascontiguousarray(inputs["mlpW2"], dtype=np.float32),
        "g1": np.asarray(inputs["ln1_g"], np.float32).reshape(1, D),
        "b1": np.asarray(inputs["ln1_b"], np.float32).reshape(1, D),
        "g2": np.asarray(inputs["ln2_g"], np.float32).reshape(1, D),
        "b2": np.asarray(inputs["ln2_b"], np.float32).reshape(1, D),
        "ident": np.eye(128, dtype=np.float32),
    }
    in_maps = []
    for c in range(NCORES):
        r0 = ROWS_PER_CORE * c + A  # first global source row of this core's range
        srcpad = np.zeros((SR, D), np.float32)
        g_lo = max(0, r0) * WW
        g_hi = min(HH, r0 + (B - A)) * WW
        if g_hi > g_lo:
            l_lo = g_lo - r0 * WW
            srcpad[l_lo : l_lo + (g_hi - g_lo)] = src[g_lo:g_hi]
        # coordT with sentinel y=-1000 on padded rows (forces mask=0)
        rows = r0 + np.arange(SR) // WW
        ys = np.where((rows >= 0) & (rows < HH), rows, -1000).astype(np.float32)
        xsc = (np.arange(SR) % WW).astype(np.float32)
        coordT = np.stack([xsc, ys, np.ones(SR, np.float32)], 0)
        in_maps.append(
            dict(
                common,
                xs=np.ascontiguousarray(x[c * LC : (c + 1) * LC]),
                srcpad=srcpad,
                linesS=np.ascontiguousarray(
                    lines_scaled[c * LC : (c + 1) * LC].T
                ),
                coordT=np.ascontiguousarray(coordT),
            )
        )
    return SR, wins, in_maps


def kernel(**inputs):
    from concourse.bass_utils import run_bass_kernel_spmd

    SR, wins, in_maps = _prepare(inputs)
    key = (SR, tuple(wins))
    if key not in _CACHE:
        _CACHE[key] = _build_program(SR, wins)
    nc = _CACHE[key]
    res = run_bass_kernel_spmd(nc, in_maps, core_ids=list(range(NCORES)))
    out = np.concatenate([res.results[c]["y"] for c in range(NCORES)], axis=0)
    return out.reshape(1, L, D).astype(np.float32)

